# revision 5
# baseline (speedup 1.0000x reference)
import math
from contextlib import ExitStack

import numpy as np
import ml_dtypes

import concourse.bass as bass
import concourse.tile as tile
from concourse import bacc, mybir
from concourse import bass_utils
from concourse.masks import make_identity

BF16 = ml_dtypes.bfloat16
NCORES = 8
NUM_ENT = 100000
NUM_REL = 400
D_IN = 200
D_OUT = 400
E = 600000
HALF = E // 2
B = 1024
P = 128
SHARD = 12544          # 98*128, ReduceScatter shard rows per core
AGG_ROWS = SHARD * 8   # 100352
TRASH = NUM_ENT        # trash agg row
VS = 500               # decoder column slice
NV = 12500             # entities per core for decoder
BN_EPS = 1e-5

F32 = mybir.dt.float32
I32 = mybir.dt.int32
BF = mybir.dt.bfloat16


def _circ(vec):
    # ct[k, j] = vec[(j+k) % D_IN], returned as [2, 128, 200] (k chunks, zero pad)
    d = D_IN
    k = np.arange(d)[:, None]
    j = np.arange(d)[None, :]
    m = vec[(j + k) % d]  # [200, 200]
    out = np.zeros((2, P, d), np.float32)
    out[0, :P] = m[:P]
    out[1, : d - P] = m[P:]
    return out


def _pad2(w):
    # [200, 400] -> [2, 128, 400] zero padded on k
    out = np.zeros((2, P, D_OUT), np.float32)
    out[0] = w[:P]
    out[1, : D_IN - P] = w[P:]
    return out


def _prep(inputs):
    src = np.asarray(inputs["src"]).astype(np.int64)
    dst = np.asarray(inputs["dst"]).astype(np.int64)
    et = np.asarray(inputs["edge_type"]).astype(np.int64)
    norm = np.asarray(inputs["edge_norm"]).astype(np.float32)

    dirs = (np.arange(E) >= HALF).astype(np.int64)

    # group edges by (dir, rel); greedy assign groups to cores per dir
    per_core_groups = [[[], []] for _ in range(NCORES)]  # [core][dir] -> list of arrays
    for d in range(2):
        groups = []
        for r in range(NUM_REL):
            idx = np.nonzero((dirs == d) & (et == r))[0]
            groups.append(idx)
        groups.sort(key=lambda a: -len(a))
        loads = [0] * NCORES
        for g in groups:
            c = int(np.argmin(loads))
            loads[c] += len(g)
            per_core_groups[c][d].append(g)
    for c in range(NCORES):
        for d in range(2):
            per_core_groups[c][d].sort(key=lambda a: -len(a))

    NS = [max(len(per_core_groups[c][d]) for c in range(NCORES)) for d in range(2)]
    TPG = []
    for d in range(2):
        tp = []
        for i in range(NS[d]):
            mx = 0
            for c in range(NCORES):
                gl = per_core_groups[c][d]
                if i < len(gl):
                    mx = max(mx, (len(gl[i]) + P - 1) // P)
            tp.append(mx)
        TPG.append(tp)
    T1 = sum(TPG[0]) + sum(TPG[1])

    # phase-2 packing per core: edges sorted by dst, runs packed into 128-tiles
    p2 = []  # per core: (tile_rows list of length T2*128 with edge id or -1, seg, voutrows)
    T2s = []
    for c in range(NCORES):
        eids = np.concatenate([g for d in range(2) for g in per_core_groups[c][d]]
                              ) if any(len(g) for d in range(2) for g in per_core_groups[c][d]) else np.zeros(0, np.int64)
        order = np.argsort(dst[eids], kind="stable")
        eids = eids[order]
        tiles = []  # list of (list_of_eids, list_of_dsts)
        cur_e, cur_d = [], []
        i = 0
        n = len(eids)
        while i < n:
            j = i
            v = dst[eids[i]]
            while j < n and dst[eids[j]] == v:
                j += 1
            run = j - i
            assert run <= P, "dst degree > 128"
            if len(cur_e) + run > P:
                tiles.append((cur_e, cur_d))
                cur_e, cur_d = [], []
            cur_e.extend(eids[i:j].tolist())
            cur_d.append(int(v))
            i = j
        if cur_e:
            tiles.append((cur_e, cur_d))
        p2.append(tiles)
        T2s.append(len(tiles))
    T = max(T1, max(T2s))

    # build per-core arrays
    data = []
    for c in range(NCORES):
        tiles = p2[c]
        seg = np.full((T * P, 1), 127, np.float32)
        vout = np.full((T * P, 1), TRASH, np.int32)
        e2q = {}
        pad_q = []
        for t in range(T):
            if t < len(tiles):
                te, td = tiles[t]
                for p, e in enumerate(te):
                    e2q[e] = t * P + p
                # seg ids
                pos = 0
                d2l = {}
                for li, v in enumerate(td):
                    d2l[v] = li
                    vout[t * P + li, 0] = v
                for p, e in enumerate(te):
                    seg[t * P + p, 0] = d2l[int(dst[e])]
                for p in range(len(te), P):
                    pad_q.append(t * P + p)
            else:
                for p in range(P):
                    pad_q.append(t * P + p)
        pad_q.reverse()

        srcA = np.zeros((T * P, 1), np.int32)
        nrmA = np.zeros((T * P, 1), np.float32)
        mposA = np.zeros((T * P, 1), np.int32)
        ct_all = np.zeros((NS[0] + NS[1] + 1, 2, P, D_IN), np.float32)
        rel_emb = np.asarray(inputs["rel_emb"]).astype(np.float32)
        q = 0
        for d in range(2):
            gl = per_core_groups[c][d]
            base = 0 if d == 0 else NS[0]
            for i in range(NS[d]):
                if i < len(gl) and len(gl[i]):
                    r = int(et[gl[i][0]])
                    ct_all[base + i] = _circ(rel_emb[r])
                nreal = len(gl[i]) if i < len(gl) else 0
                for k in range(TPG[d][i] * P):
                    if k < nreal:
                        e = int(gl[i][k])
                        srcA[q, 0] = src[e]
                        nrmA[q, 0] = norm[e]
                        mposA[q, 0] = e2q[e]
                    else:
                        srcA[q, 0] = 0
                        nrmA[q, 0] = 0.0
                        mposA[q, 0] = pad_q.pop()
                    q += 1
        while q < T * P:
            srcA[q, 0] = 0
            nrmA[q, 0] = 0.0
            mposA[q, 0] = pad_q.pop()
            q += 1
        assert not pad_q, f"pad mismatch {len(pad_q)}"
        ct_all[NS[0] + NS[1]] = _circ(
            np.asarray(inputs["loop_rel"]).astype(np.float32)[0])

        # x pipeline indices
        vent = np.zeros((SHARD, 1), np.int32)
        vmask = np.zeros((SHARD, 1), np.float32)
        for i in range(SHARD):
            v = c * SHARD + i
            if v < NUM_ENT:
                vent[i, 0] = v
                vmask[i, 0] = 1.0
        triples = np.asarray(inputs["triples"]).astype(np.int64)
        head = triples[:, 0]
        hidx = np.zeros((B, 1), np.int32)
        hmask = np.zeros((B, 1), np.float32)
        own = head // SHARD == c
        hidx[own, 0] = (head[own] - c * SHARD).astype(np.int32)
        hmask[own, 0] = 1.0

        data.append(dict(srcA=srcA, nrmA=nrmA, mposA=mposA, seg=seg, vout=vout,
                         ct_all=ct_all.astype(BF16), vent=vent, vmask=vmask,
                         hidx=hidx, hmask=hmask))
    return data, NS, TPG, T


def _build(NS, TPG, T):
    nc = bacc.Bacc("TRN2", target_bir_lowering=False, debug=False,
                   num_devices=NCORES)
    NSLOT = NS[0] + NS[1] + 1
    ent = nc.dram_tensor("ent", [NUM_ENT, D_IN], F32, kind="ExternalInput")
    ct_all = nc.dram_tensor("ct_all", [NSLOT, 2, P, D_IN], BF, kind="ExternalInput")
    w_in = nc.dram_tensor("w_in", [2, P, D_OUT], BF, kind="ExternalInput")
    w_out = nc.dram_tensor("w_out", [2, P, D_OUT], BF, kind="ExternalInput")
    w_loop = nc.dram_tensor("w_loop", [2, P, D_OUT], BF, kind="ExternalInput")
    relT = nc.dram_tensor("relT", [2, P, NUM_REL], BF, kind="ExternalInput")
    wrel = nc.dram_tensor("wrel", [2, P, D_OUT], BF, kind="ExternalInput")
    srcA = nc.dram_tensor("srcA", [T * P, 1], I32, kind="ExternalInput")
    nrmA = nc.dram_tensor("nrmA", [T * P, 1], F32, kind="ExternalInput")
    mposA = nc.dram_tensor("mposA", [T * P, 1], I32, kind="ExternalInput")
    segA = nc.dram_tensor("segA", [T * P, 1], F32, kind="ExternalInput")
    voutA = nc.dram_tensor("voutA", [T * P, 1], I32, kind="ExternalInput")
    ventA = nc.dram_tensor("ventA", [SHARD, 1], I32, kind="ExternalInput")
    vmaskA = nc.dram_tensor("vmaskA", [SHARD, 1], F32, kind="ExternalInput")
    hidxA = nc.dram_tensor("hidxA", [B, 1], I32, kind="ExternalInput")
    hmaskA = nc.dram_tensor("hmaskA", [B, 1], F32, kind="ExternalInput")
    relaA = nc.dram_tensor("relaA", [B, 1], I32, kind="ExternalInput")
    gamma = nc.dram_tensor("gamma", [1, D_OUT], F32, kind="ExternalInput")
    beta = nc.dram_tensor("beta", [1, D_OUT], F32, kind="ExternalInput")
    embw = nc.dram_tensor("embw", [4, P, NV], BF, kind="ExternalInput")
    ebias = nc.dram_tensor("ebias", [1, NV], BF, kind="ExternalInput")
    score = nc.dram_tensor("score", [B, NV], F32, kind="ExternalOutput")

    with tile.TileContext(nc) as tc, ExitStack() as ctx:
        sb = ctx.enter_context(tc.tile_pool(name="sb", bufs=4))
        cst = ctx.enter_context(tc.tile_pool(name="cst", bufs=1))
        pp = ctx.enter_context(tc.tile_pool(name="pp", bufs=3, space="PSUM"))
        ppb = ctx.enter_context(tc.tile_pool(name="ppb", bufs=1, space="PSUM"))
        pst = ctx.enter_context(tc.tile_pool(name="pst", bufs=1, space="PSUM"))
        dram = ctx.enter_context(tc.tile_pool(name="dram", bufs=1, space="DRAM"))

        msg_d = dram.tile([T * P, D_OUT], BF, tag="msg_d")
        pagg = dram.tile([AGG_ROWS, D_OUT], BF, tag="pagg")
        ragg = dram.tile([SHARD, D_OUT], BF, tag="ragg")
        x_d = dram.tile([SHARD, D_OUT], BF, tag="x_d")
        hx_l = dram.tile([B, D_OUT], BF, tag="hx_l")
        hx_f = dram.tile([B, D_OUT], BF, tag="hx_f")
        r_d = dram.tile([NUM_REL, D_OUT], BF, tag="r_d")
        st_l = dram.tile([1, 2 * D_OUT], F32, tag="st_l")
        st_f = dram.tile([1, 2 * D_OUT], F32, tag="st_f")

        identf = cst.tile([P, P], F32, tag="identf")
        make_identity(nc, identf[:])
        identb = cst.tile([P, P], BF, tag="identb")
        make_identity(nc, identb[:])
        iota_i = cst.tile([P, P], I32, tag="iota_i")
        nc.gpsimd.iota(iota_i[:], [[1, P]], base=0, channel_multiplier=0)
        iota_f = cst.tile([P, P], F32, tag="iota_f")
        nc.vector.tensor_copy(iota_f[:], iota_i[:])
        ones_r = cst.tile([1, P], BF, tag="ones_r")
        nc.gpsimd.memset(ones_r[:], 1.0)
        zero_sb = cst.tile([P, 3200], BF, tag="zero_sb")
        nc.gpsimd.memset(zero_sb[:], 0.0)

        # zero partial agg (100352*400 bf16 = 80MB)
        rows_per = 1024  # [128, 3200] covers 1024 rows of 400
        for i in range(AGG_ROWS // rows_per):
            nc.sync.dma_start(
                bass.AP(pagg.tensor, i * rows_per * D_OUT, [[3200, P], [1, 3200]]),
                zero_sb[:])

        # W tiles resident
        def load_w(t):
            w = cst.tile([P, 2 * D_OUT], BF, tag=f"w{t.name}")
            nc.sync.dma_start(w[:, 0:D_OUT], t[0, :, :])
            nc.sync.dma_start(w[:, D_OUT:2 * D_OUT], t[1, :, :])
            return w
        w_in_sb = load_w(w_in)
        w_out_sb = load_w(w_out)
        w_loop_sb = load_w(w_loop)

        def build_mt(slot, w_sb, scale=1.0):
            ct = sb.tile([P, 2 * D_IN], BF, tag="ct")
            nc.sync.dma_start(ct[:, 0:D_IN], ct_all[slot, 0, :, :])
            nc.sync.dma_start(ct[:, D_IN:2 * D_IN], ct_all[slot, 1, :, :])
            mt = sb.tile([P, 2 * D_OUT], BF, tag="mt")
            for jc in range(2):
                js = P if jc == 0 else D_IN - P
                mps = pp.tile([P, D_OUT], F32, tag="mm", space="PSUM")
                for kc in range(2):
                    nc.tensor.matmul(
                        out=mps[:js, :],
                        lhsT=ct[:, kc * D_IN + jc * P:kc * D_IN + jc * P + js],
                        rhs=w_sb[:, kc * D_OUT:(kc + 1) * D_OUT],
                        start=(kc == 0), stop=(kc == 1))
                nc.scalar.activation(mt[:js, jc * D_OUT:(jc + 1) * D_OUT],
                                     mps[:js, :],
                                     mybir.ActivationFunctionType.Copy, scale=scale)
            return mt

        # ---------------- phase 1: messages ----------------
        def p1_tile(q, mt):
            it = sb.tile([P, 1], I32, tag="it")
            nc.sync.dma_start(it[:], srcA[q * P:(q + 1) * P, :])
            nt = sb.tile([P, 1], F32, tag="nt")
            nc.sync.dma_start(nt[:], nrmA[q * P:(q + 1) * P, :])
            n3 = sb.tile([P, 1], F32, tag="n3")
            nc.vector.tensor_scalar_mul(n3[:], nt[:], 1.0 / 3.0)
            pt = sb.tile([P, 1], I32, tag="pt")
            nc.sync.dma_start(pt[:], mposA[q * P:(q + 1) * P, :])
            a = sb.tile([P, D_IN], F32, tag="a")
            nc.gpsimd.indirect_dma_start(
                out=a[:], out_offset=None, in_=ent[:, :],
                in_offset=bass.IndirectOffsetOnAxis(ap=it[:, :1], axis=0))
            at = sb.tile([P, 2 * P], BF, tag="at")
            for jc in range(2):
                js = P if jc == 0 else D_IN - P
                tp = pp.tile([P, P], F32, tag="mm", space="PSUM")
                nc.tensor.transpose(out=tp[:js, :], in_=a[:, jc * P:jc * P + js],
                                    identity=identf[:])
                nc.vector.tensor_copy(at[:js, jc * P:(jc + 1) * P], tp[:js, :])
            mps = pp.tile([P, D_OUT], F32, tag="mm", space="PSUM")
            for jc in range(2):
                js = P if jc == 0 else D_IN - P
                nc.tensor.matmul(out=mps[:], lhsT=at[:js, jc * P:(jc + 1) * P],
                                 rhs=mt[:js, jc * D_OUT:(jc + 1) * D_OUT],
                                 start=(jc == 0), stop=(jc == 1))
            mb = sb.tile([P, D_OUT], BF, tag="mb")
            nc.scalar.activation(mb[:], mps[:],
                                 mybir.ActivationFunctionType.Copy, scale=n3[:, :1])
            nc.gpsimd.indirect_dma_start(
                out=msg_d[:, :],
                out_offset=bass.IndirectOffsetOnAxis(ap=pt[:, :1], axis=0),
                in_=mb[:], in_offset=None)

        q = 0
        mt = None
        for d in range(2):
            w_sb = w_in_sb if d == 0 else w_out_sb
            base = 0 if d == 0 else NS[0]
            for i in range(NS[d]):
                if TPG[d][i] == 0:
                    continue
                mt = build_mt(base + i, w_sb)
                for _ in range(TPG[d][i]):
                    p1_tile(q, mt)
                    q += 1
        while q < T:
            p1_tile(q, mt)
            q += 1

        # ---------------- phase 2: segment sum ----------------
        for t in range(T):
            mrows = sb.tile([P, D_OUT], BF, tag="mrows")
            nc.sync.dma_start(mrows[:], msg_d[t * P:(t + 1) * P, :])
            sg = sb.tile([P, 1], F32, tag="sg")
            nc.sync.dma_start(sg[:], segA[t * P:(t + 1) * P, :])
            vo = sb.tile([P, 1], I32, tag="vo")
            nc.sync.dma_start(vo[:], voutA[t * P:(t + 1) * P, :])
            S = sb.tile([P, P], BF, tag="S")
            nc.vector.tensor_scalar(S[:], iota_f[:], sg[:, :1], None,
                                    op0=mybir.AluOpType.is_equal)
            ps = pp.tile([P, D_OUT], F32, tag="mm", space="PSUM")
            nc.tensor.matmul(out=ps[:], lhsT=S[:], rhs=mrows[:],
                             start=True, stop=True)
            ab = sb.tile([P, D_OUT], BF, tag="ab")
            nc.scalar.activation(ab[:], ps[:],
                                 mybir.ActivationFunctionType.Copy)
            nc.gpsimd.indirect_dma_start(
                out=pagg[:, :],
                out_offset=bass.IndirectOffsetOnAxis(ap=vo[:, :1], axis=0),
                in_=ab[:], in_offset=None)

        # reduce-scatter partial agg -> local shard
        nc.gpsimd.collective_compute(
            "ReduceScatter", mybir.AluOpType.add,
            replica_groups=[list(range(NCORES))],
            ins=[pagg.opt()], outs=[ragg.opt()])

        # ---------------- x = agg + loop, stats ----------------
        ml = build_mt(NS[0] + NS[1], w_loop_sb, scale=1.0 / 3.0)
        ps1 = pst.tile([1, D_OUT], F32, tag="ps1", space="PSUM")
        ps2 = pst.tile([1, D_OUT], F32, tag="ps2", space="PSUM")
        NT = SHARD // P
        for t in range(NT):
            ve = sb.tile([P, 1], I32, tag="ve")
            nc.sync.dma_start(ve[:], ventA[t * P:(t + 1) * P, :])
            vm = sb.tile([P, 1], F32, tag="vm")
            nc.sync.dma_start(vm[:], vmaskA[t * P:(t + 1) * P, :])
            vmb = sb.tile([P, 1], BF, tag="vmb")
            nc.vector.tensor_copy(vmb[:], vm[:])
            av = sb.tile([P, D_IN], F32, tag="a")
            nc.gpsimd.indirect_dma_start(
                out=av[:], out_offset=None, in_=ent[:, :],
                in_offset=bass.IndirectOffsetOnAxis(ap=ve[:, :1], axis=0))
            at = sb.tile([P, 2 * P], BF, tag="at")
            for jc in range(2):
                js = P if jc == 0 else D_IN - P
                tp = pp.tile([P, P], F32, tag="mm", space="PSUM")
                nc.tensor.transpose(out=tp[:js, :], in_=av[:, jc * P:jc * P + js],
                                    identity=identf[:])
                nc.vector.tensor_copy(at[:js, jc * P:(jc + 1) * P], tp[:js, :])
            lp = pp.tile([P, D_OUT], F32, tag="mm", space="PSUM")
            for jc in range(2):
                js = P if jc == 0 else D_IN - P
                nc.tensor.matmul(out=lp[:], lhsT=at[:js, jc * P:(jc + 1) * P],
                                 rhs=ml[:js, jc * D_OUT:(jc + 1) * D_OUT],
                                 start=(jc == 0), stop=(jc == 1))
            ag = sb.tile([P, D_OUT], BF, tag="ag")
            nc.sync.dma_start(ag[:], ragg[t * P:(t + 1) * P, :])
            xb = sb.tile([P, D_OUT], BF, tag="xb")
            nc.vector.tensor_add(xb[:], ag[:], lp[:])
            nc.sync.dma_start(x_d[t * P:(t + 1) * P, :], xb[:])
            xs = sb.tile([P, D_OUT], BF, tag="xs")
            nc.vector.tensor_mul(xs[:], xb[:], xb[:])
            nc.tensor.matmul(out=ps1[:], lhsT=vmb[:], rhs=xb[:],
                             start=(t == 0), stop=(t == NT - 1))
            nc.tensor.matmul(out=ps2[:], lhsT=vmb[:], rhs=xs[:],
                             start=(t == 0), stop=(t == NT - 1))
        stl = sb.tile([1, 2 * D_OUT], F32, tag="stl")
        nc.vector.tensor_copy(stl[:, 0:D_OUT], ps1[:])
        nc.vector.tensor_copy(stl[:, D_OUT:2 * D_OUT], ps2[:])
        nc.sync.dma_start(st_l[:, :], stl[:])
        nc.gpsimd.collective_compute(
            "AllReduce", mybir.AluOpType.add,
            replica_groups=[list(range(NCORES))],
            ins=[st_l.opt()], outs=[st_f.opt()])

        # s = gamma / sqrt(var+eps), b = beta - mean*s
        stf = sb.tile([1, 2 * D_OUT], F32, tag="stf")
        nc.sync.dma_start(stf[:], st_f[:, :])
        mean = sb.tile([1, D_OUT], F32, tag="mean")
        nc.vector.tensor_scalar_mul(mean[:], stf[:, 0:D_OUT], 1.0 / NUM_ENT)
        var = sb.tile([1, D_OUT], F32, tag="var")
        nc.vector.tensor_scalar_mul(var[:], stf[:, D_OUT:2 * D_OUT], 1.0 / NUM_ENT)
        m2 = sb.tile([1, D_OUT], F32, tag="m2")
        nc.vector.tensor_mul(m2[:], mean[:], mean[:])
        nc.vector.tensor_sub(var[:], var[:], m2[:])
        nc.vector.tensor_scalar_add(var[:], var[:], BN_EPS)
        sd = sb.tile([1, D_OUT], F32, tag="sd")
        nc.scalar.sqrt(sd[:], var[:])
        rsd = sb.tile([1, D_OUT], F32, tag="rsd")
        nc.vector.reciprocal(rsd[:], sd[:])
        gm = sb.tile([1, D_OUT], F32, tag="gm")
        nc.sync.dma_start(gm[:], gamma[:, :])
        bt = sb.tile([1, D_OUT], F32, tag="bt")
        nc.sync.dma_start(bt[:], beta[:, :])
        sv = sb.tile([1, D_OUT], BF, tag="sv")
        nc.vector.tensor_mul(sv[:], gm[:], rsd[:])
        svf = sb.tile([1, D_OUT], F32, tag="svf")
        nc.vector.tensor_copy(svf[:], sv[:])
        bv = sb.tile([1, D_OUT], BF, tag="bv")
        ms = sb.tile([1, D_OUT], F32, tag="ms")
        nc.vector.tensor_mul(ms[:], mean[:], svf[:])
        nc.vector.tensor_sub(bv[:], bt[:], ms[:])
        # broadcast to [128, 400]
        sR = sb.tile([P, D_OUT], BF, tag="sR")
        bR = sb.tile([P, D_OUT], BF, tag="bR")
        for srcv, dstv in ((sv, sR), (bv, bR)):
            pb = pp.tile([P, D_OUT], F32, tag="mm", space="PSUM")
            nc.tensor.matmul(out=pb[:], lhsT=ones_r[:1, :], rhs=srcv[:1, :],
                             start=True, stop=True)
            nc.vector.tensor_copy(dstv[:], pb[:])

        # r = rel_emb @ w_rel -> r_d
        wr = load_w(wrel)
        rT = cst.tile([P, 2 * NUM_REL], BF, tag="rT")
        nc.sync.dma_start(rT[:, 0:NUM_REL], relT[0, :, :])
        nc.sync.dma_start(rT[:, NUM_REL:2 * NUM_REL], relT[1, :, :])
        for mc in range(4):
            pr = pp.tile([P, D_OUT], F32, tag="mm", space="PSUM")
            for kc in range(2):
                nc.tensor.matmul(
                    out=pr[:100, :],
                    lhsT=rT[:, kc * NUM_REL + mc * 100:kc * NUM_REL + (mc + 1) * 100],
                    rhs=wr[:, kc * D_OUT:(kc + 1) * D_OUT],
                    start=(kc == 0), stop=(kc == 1))
            rb_ = sb.tile([P, D_OUT], BF, tag="rb_")
            nc.scalar.activation(rb_[:100, :], pr[:100, :],
                                 mybir.ActivationFunctionType.Copy)
            nc.sync.dma_start(r_d[mc * 100:(mc + 1) * 100, :], rb_[:100, :])

        # heads: gather x rows, BN+tanh, mask, assemble
        for t in range(B // P):
            hi = sb.tile([P, 1], I32, tag="hi")
            nc.sync.dma_start(hi[:], hidxA[t * P:(t + 1) * P, :])
            hm = sb.tile([P, 1], F32, tag="hm")
            nc.sync.dma_start(hm[:], hmaskA[t * P:(t + 1) * P, :])
            xg = sb.tile([P, D_OUT], BF, tag="xg")
            nc.gpsimd.indirect_dma_start(
                out=xg[:], out_offset=None, in_=x_d[:, :],
                in_offset=bass.IndirectOffsetOnAxis(ap=hi[:, :1], axis=0))
            xn = sb.tile([P, D_OUT], BF, tag="xn")
            nc.vector.tensor_mul(xn[:], xg[:], sR[:])
            nc.vector.tensor_add(xn[:], xn[:], bR[:])
            xt = sb.tile([P, D_OUT], BF, tag="xt")
            nc.scalar.activation(xt[:], xn[:], mybir.ActivationFunctionType.Tanh)
            hx = sb.tile([P, D_OUT], BF, tag="hx")
            nc.vector.tensor_scalar_mul(hx[:], xt[:], hm[:, :1])
            nc.sync.dma_start(hx_l[t * P:(t + 1) * P, :], hx[:])
        nc.gpsimd.collective_compute(
            "AllReduce", mybir.AluOpType.add,
            replica_groups=[list(range(NCORES))],
            ins=[hx_l.opt()], outs=[hx_f.opt()])

        # obj = hx * r[rela]; objT chunks
        objT = []
        for t in range(B // P):
            ra = sb.tile([P, 1], I32, tag="ra")
            nc.sync.dma_start(ra[:], relaA[t * P:(t + 1) * P, :])
            rr = sb.tile([P, D_OUT], BF, tag="rr")
            nc.gpsimd.indirect_dma_start(
                out=rr[:], out_offset=None, in_=r_d[:, :],
                in_offset=bass.IndirectOffsetOnAxis(ap=ra[:, :1], axis=0))
            hh = sb.tile([P, D_OUT], BF, tag="hh")
            nc.sync.dma_start(hh[:], hx_f[t * P:(t + 1) * P, :])
            ob = sb.tile([P, D_OUT], BF, tag="ob")
            nc.vector.tensor_mul(ob[:], hh[:], rr[:])
            row = []
            for fc in range(4):
                tp = ppb.tile([P, P], BF, tag="mmb", space="PSUM")
                nc.tensor.transpose(out=tp[:100, :],
                                    in_=ob[:, fc * 100:(fc + 1) * 100],
                                    identity=identb[:])
                ot = cst.tile([100, P], BF, tag=f"ot{t}_{fc}")
                nc.vector.tensor_copy(ot[:], tp[:100, :])
                row.append(ot)
            objT.append(row)

        # decoder
        for v in range(NV // VS):
            ew = []
            for fc in range(4):
                w = sb.tile([P, VS], BF, tag="ew")
                nc.sync.dma_start(w[:], embw[fc, :, v * VS:(v + 1) * VS])
                ew.append(w)
            eb = sb.tile([1, VS], BF, tag="eb")
            nc.sync.dma_start(eb[:], ebias[:, v * VS:(v + 1) * VS])
            for t in range(B // P):
                pd = pp.tile([P, VS], F32, tag="mm", space="PSUM")
                for fc in range(4):
                    nc.tensor.matmul(out=pd[:], lhsT=objT[t][fc][:, :],
                                     rhs=ew[fc][:100, :],
                                     start=(fc == 0), stop=False)
                nc.tensor.matmul(out=pd[:], lhsT=ones_r[:1, :], rhs=eb[:1, :],
                                 start=False, stop=True)
                sc = sb.tile([P, VS], F32, tag="sc")
                nc.scalar.activation(sc[:], pd[:],
                                     mybir.ActivationFunctionType.Sigmoid)
                nc.sync.dma_start(score[t * P:(t + 1) * P, v * VS:(v + 1) * VS],
                                  sc[:])
    nc.compile()
    return nc


def kernel(**inputs):
    data, NS, TPG, T = _prep(inputs)
    nc = _build(NS, TPG, T)

    ent = np.ascontiguousarray(np.asarray(inputs["ent_emb"], np.float32))
    w_in = _pad2(np.asarray(inputs["in_w"], np.float32)).astype(BF16)
    w_out = _pad2(np.asarray(inputs["out_w"], np.float32)).astype(BF16)
    w_loop = _pad2(np.asarray(inputs["loop_w"], np.float32)).astype(BF16)
    wrel = _pad2(np.asarray(inputs["w_rel"], np.float32)).astype(BF16)
    relT = np.zeros((2, P, NUM_REL), np.float32)
    re = np.asarray(inputs["rel_emb"], np.float32).T  # [200, 400]
    relT[0] = re[:P]
    relT[1, : D_IN - P] = re[P:]
    relT = relT.astype(BF16)
    gamma = np.asarray(inputs["bn_gamma"], np.float32).reshape(1, D_OUT)
    beta = np.asarray(inputs["bn_beta"], np.float32).reshape(1, D_OUT)
    rela = np.asarray(inputs["triples"])[:, 1].astype(np.int32).reshape(B, 1)
    ew_full = np.asarray(inputs["emb_ent_w"], np.float32)  # [100000, 400]
    ebias_full = np.asarray(inputs["ent_bias"], np.float32)

    in_maps = []
    for c in range(NCORES):
        d = data[c]
        sl = slice(c * NV, (c + 1) * NV)
        embw = np.zeros((4, P, NV), np.float32)
        ewT = ew_full[sl].T  # [400, 12500]
        for fc in range(4):
            embw[fc, :100] = ewT[fc * 100:(fc + 1) * 100]
        in_maps.append({
            "ent": ent, "ct_all": d["ct_all"], "w_in": w_in, "w_out": w_out,
            "w_loop": w_loop, "relT": relT, "wrel": wrel,
            "srcA": d["srcA"], "nrmA": d["nrmA"], "mposA": d["mposA"],
            "segA": d["seg"], "voutA": d["vout"], "ventA": d["vent"],
            "vmaskA": d["vmask"], "hidxA": d["hidx"], "hmaskA": d["hmask"],
            "relaA": rela, "gamma": gamma, "beta": beta,
            "embw": embw.astype(BF16),
            "ebias": ebias_full[sl].reshape(1, NV).astype(BF16),
        })

    res = bass_utils.run_bass_kernel_spmd(nc, in_maps,
                                          core_ids=list(range(NCORES)))
    out = np.concatenate([res.results[c]["score"] for c in range(NCORES)],
                         axis=1)
    return out.astype(np.float32)
